# revision 4
# baseline (speedup 1.0000x reference)
"""Block-sliding-window attention (trunk 32 queries, window 128 keys, d=64)
for [1, 16, 16384, 64] f32 inputs, distributed over 8 NeuronCores (2 heads each).

Algorithm (per head, transposed-scores layout):
  - keys are processed in 128 chunks of 128 consecutive positions
  - chunk c pairs with 8 query trunks t in [4c-2, 4c+6); scores are computed
    transposed: sT[kpos 128, q 256] = kT_chunk.T-contracted-over-d @ qT_cols
  - exp (no max subtraction; scores are bounded, f32/bf16 exp safe)
  - band mask applied as elementwise multiply with a constant [128, 256] 0/1 tile
  - AV rides two bf16 matmuls (old/new query half); V carries an appended
    ones-column so the softmax denominator accumulates in the same PSUM tile
  - normalization: reciprocal of the ones-column, broadcast multiply

Host-side prep (free; only HW time counts): Q/K transposed to [d, seq] and
packed two-halves-in-128-partitions for full-bandwidth DMA, V packed to the
[128, chunk, 65] ones-augmented layout, output un-permuted from the block
layout the device writes.
"""
import os
import numpy as np
import ml_dtypes

import concourse.bass as bass
import concourse.tile as tile
from concourse import bacc, mybir
from concourse.bass import ds
from concourse.bass_utils import run_bass_kernel_spmd

F32 = mybir.dt.float32
F32R = mybir.dt.float32r
BF16 = mybir.dt.bfloat16

N = 16384
D = 64
NQ = 32          # trunk size
NK = 128         # window size
PAD = (NK - NQ) // 2      # 48 key halo, but the q-col pad is 64 (2 trunks)
C = N // 128     # 128 key chunks
B = C + 1        # 129 query blocks of 128 rows, block b = seq [128b-64, 128b+64)
H_PER_CORE = 2
N_CORES = 8

# qT padded to 16512 cols (64 zero-cols each side); packed halves overlap so
# every chunk's 256-col q-slice stays inside one half.
QT_COLS = 8448   # half A = padded cols [0, 8448), half B = [8064, 16512)
QT_B_OFF = 8064
KT_COLS = 8192   # half A = cols [0, 8192), half B = [8192, 16384)

QK_DTYPE = os.environ.get("QK_DTYPE", "f32r")  # "f32r" | "bf16"

LAST_EXEC_TIME_NS = None
LAST_RESULTS = None


def _build_mask() -> np.ndarray:
    """mask[kk, j] = 1 iff 32g-112 <= kk < 32g+16, g = j//32."""
    kk = np.arange(128)[:, None]
    g = np.arange(256)[None, :] // 32
    valid = (kk >= 32 * g - 112) & (kk < 32 * g + 16)
    return valid.astype(ml_dtypes.bfloat16)


def build_nc():
    dt_qk = F32R if QK_DTYPE == "f32r" else BF16
    nc = bacc.Bacc(None, target_bir_lowering=False)

    qt_ext = nc.declare_dram_parameter("qt", [H_PER_CORE, 128, QT_COLS], dt_qk, isOutput=False)
    kt_ext = nc.declare_dram_parameter("kt", [H_PER_CORE, 128, KT_COLS], dt_qk, isOutput=False)
    v_ext = nc.declare_dram_parameter("v65", [H_PER_CORE, 128, C * 65], BF16, isOutput=False)
    m_ext = nc.declare_dram_parameter("mask", [128, 256], BF16, isOutput=False)
    out_ext = nc.declare_dram_parameter("out", [H_PER_CORE, 128, B * 64], F32, isOutput=True)

    with tile.TileContext(nc) as tc:
        with (
            tc.tile_pool(name="inputs", bufs=2) as inputs,
            tc.tile_pool(name="singles", bufs=1) as singles,
            tc.tile_pool(name="at", bufs=3) as at_pool,
            tc.tile_pool(name="st", bufs=2) as st_pool,
            tc.tile_pool(name="rt", bufs=2) as rt_pool,
            tc.tile_pool(name="ps_s", bufs=3, space="PSUM") as ps_s,
            tc.tile_pool(name="ps_o", bufs=2, space="PSUM") as ps_o,
        ):
            mask_t = singles.tile([128, 256], BF16)
            nc.sync.dma_start(out=mask_t, in_=m_ext[:, :])

            for h in range(H_PER_CORE):
                qt_t = inputs.tile([128, QT_COLS], dt_qk, tag="qt")
                kt_t = inputs.tile([128, KT_COLS], dt_qk, tag="kt")
                v_t = inputs.tile([128, C * 65], BF16, tag="v")
                nc.sync.dma_start(out=qt_t, in_=qt_ext[h])
                nc.sync.dma_start(out=kt_t, in_=kt_ext[h])
                nc.sync.dma_start(out=v_t, in_=v_ext[h])

                po = {}   # batch J -> psum tile [128, 4, 65]
                po_touched = set()  # J values whose tile already has a start=True
                st_t = None
                for c in range(C):
                    # --- QK^T (transposed scores) ---
                    if c < 64:
                        lhs = kt_t[0:64, ds(128 * c, 128)]
                        rhs = qt_t[0:64, ds(128 * c, 256)]
                    else:
                        lhs = kt_t[64:128, ds(128 * c - KT_COLS, 128)]
                        rhs = qt_t[64:128, ds(128 * c - QT_B_OFF, 256)]
                    s_ps = ps_s.tile([128, 256], F32)
                    nc.tensor.matmul(s_ps, lhsT=lhs, rhs=rhs, start=True, stop=True)

                    # --- exp + mask ---
                    at_t = at_pool.tile([128, 256], BF16)
                    nc.scalar.activation(out=at_t, in_=s_ps, func=mybir.ActivationFunctionType.Exp)
                    nc.vector.tensor_mul(at_t, at_t, mask_t)

                    # --- AV + rowsum (ones-column in v) ---
                    j_new, j_old = (c + 1) % 4, c % 4
                    J_new, J_old = (c + 1) // 4, c // 4
                    if J_new not in po:
                        po[J_new] = ps_o.tile([128, 4, 65], F32, tag="po", name=f"po_h{h}_{J_new}")
                    if J_old not in po:
                        po[J_old] = ps_o.tile([128, 4, 65], F32, tag="po", name=f"po_h{h}_{J_old}")
                    vslice = v_t[:, ds(65 * c, 65)]
                    # start=True clears the ENTIRE psum bank (has_written bits),
                    # so only the first matmul touching each 4-block tile may
                    # set it; later slots rely on clear has_written bits to
                    # overwrite on first write and accumulate on second.
                    nc.tensor.matmul(
                        po[J_new][:, j_new, :], lhsT=at_t[:, 128:256], rhs=vslice,
                        start=(J_new not in po_touched),
                        stop=(c == C - 1), skip_group_check=True,
                    )
                    po_touched.add(J_new)
                    nc.tensor.matmul(
                        po[J_old][:, j_old, :], lhsT=at_t[:, 0:128], rhs=vslice,
                        start=(J_old not in po_touched),
                        stop=(j_old == 3), skip_group_check=True,
                    )
                    po_touched.add(J_old)

                    # --- normalize completed batch of 4 blocks ---
                    if c % 4 == 3:
                        J = c // 4            # blocks 4J..4J+3 done
                        if st_t is None:
                            st_t = st_pool.tile([128, 16, 64], F32, tag="st")
                        pj = po.pop(J)
                        rt_t = rt_pool.tile([128, 4], F32)
                        nc.vector.reciprocal(rt_t, pj[:, :, 64:65])
                        rb = bass.AP(
                            tensor=rt_t.tensor, offset=rt_t.offset,
                            ap=[rt_t.ap[0], rt_t.ap[1], [0, 64]],
                        )
                        s4 = J % 4
                        nc.vector.tensor_mul(
                            st_t[:, ds(4 * s4, 4), :], pj[:, :, 0:64], rb,
                        )
                        if J % 4 == 3:
                            G = J // 4        # blocks 16G..16G+15 staged
                            nc.sync.dma_start(
                                out=out_ext[h][:, ds(1024 * G, 1024)], in_=st_t,
                            )
                            st_t = None

                # --- tail: block 128 (batch 32, slot 0) ---
                pj = po.pop(32)
                rt_t = rt_pool.tile([128, 1], F32)
                nc.vector.reciprocal(rt_t, pj[:, 0, 64:65])
                rb = bass.AP(
                    tensor=rt_t.tensor, offset=rt_t.offset,
                    ap=[rt_t.ap[0], [0, 64]],
                )
                ot = st_pool.tile([128, 64], F32, tag="tail")
                nc.vector.tensor_mul(ot, pj[:, 0, 0:64], rb)
                nc.sync.dma_start(out=out_ext[h][:, ds(64 * 128, 64)], in_=ot)

    nc.finalize()
    return nc


_NC_CACHE = {}


def _get_nc():
    key = QK_DTYPE
    if key not in _NC_CACHE:
        _NC_CACHE[key] = build_nc()
    return _NC_CACHE[key]


def _prep_core(q2: np.ndarray, k2: np.ndarray, v2: np.ndarray, mask: np.ndarray):
    """q2/k2/v2: [2, N, D] f32 for this core's heads -> in_map dict."""
    np_qk = np.float32 if QK_DTYPE == "f32r" else ml_dtypes.bfloat16
    qt = np.zeros((H_PER_CORE, 128, QT_COLS), dtype=np_qk)
    kt = np.empty((H_PER_CORE, 128, KT_COLS), dtype=np_qk)
    v65 = np.empty((H_PER_CORE, 128, C * 65), dtype=ml_dtypes.bfloat16)
    for h in range(H_PER_CORE):
        qT = np.zeros((D, 64 + N + 64), dtype=np.float32)
        qT[:, 64:64 + N] = q2[h].T
        qt[h, 0:64] = qT[:, 0:QT_COLS].astype(np_qk)
        qt[h, 64:128] = qT[:, QT_B_OFF:QT_B_OFF + QT_COLS].astype(np_qk)
        kT = k2[h].T.astype(np_qk)
        kt[h, 0:64] = kT[:, 0:KT_COLS]
        kt[h, 64:128] = kT[:, KT_COLS:N]
        vv = np.ones((128, C, 65), dtype=ml_dtypes.bfloat16)
        vv[:, :, 0:64] = np.transpose(
            v2[h].reshape(C, 128, D), (1, 0, 2)
        ).astype(ml_dtypes.bfloat16)
        v65[h] = vv.reshape(128, C * 65)
    return {"qt": qt, "kt": kt, "v65": v65, "mask": mask}


def kernel(q: np.ndarray, k: np.ndarray, v: np.ndarray) -> np.ndarray:
    global LAST_EXEC_TIME_NS, LAST_RESULTS
    q = np.asarray(q)
    k = np.asarray(k)
    v = np.asarray(v)
    Bq, H = q.shape[0], q.shape[1]
    assert (Bq, H) == (1, 16) and q.shape[2] == N and q.shape[3] == D

    mask = _build_mask()
    in_maps = []
    for i in range(N_CORES):
        hs = slice(H_PER_CORE * i, H_PER_CORE * (i + 1))
        in_maps.append(_prep_core(q[0, hs], k[0, hs], v[0, hs], mask))

    nc = _get_nc()
    res = run_bass_kernel_spmd(nc, in_maps, core_ids=list(range(N_CORES)))
    LAST_RESULTS = res
    LAST_EXEC_TIME_NS = res.exec_time_ns

    out = np.empty((1, H, N, D), dtype=np.float32)
    for i in range(N_CORES):
        od = np.asarray(res.results[i]["out"])  # [2, 128, B*64]
        # [2, 128, B, 64] -> [2, B, 128, 64] -> [2, B*128, 64]; rows 64..64+N
        o = od.reshape(H_PER_CORE, 128, B, 64).transpose(0, 2, 1, 3).reshape(
            H_PER_CORE, B * 128, 64
        )
        out[0, H_PER_CORE * i:H_PER_CORE * (i + 1)] = o[:, 64:64 + N, :]
    return out


# revision 10
# speedup vs baseline: 1.4364x; 1.4364x over previous
"""Block-sliding-window attention (trunk 32 queries, window 128 keys, d=64)
for [1, 16, 16384, 64] f32 inputs, distributed over 8 NeuronCores (2 heads each).

Algorithm (per head, transposed-scores layout):
  - keys are processed in 128 chunks of 128 consecutive positions
  - chunk c pairs with 8 query trunks t in [4c-2, 4c+6); scores are computed
    transposed: sT[kpos 128, q 256] = kT_chunk.T-contracted-over-d @ qT_cols
  - exp (no max subtraction; scores are bounded, f32/bf16 exp safe)
  - band mask applied as elementwise multiply with a constant [128, 256] 0/1 tile
  - AV rides two bf16 matmuls (old/new query half); V carries an appended
    ones-column so the softmax denominator accumulates in the same PSUM tile
  - normalization: reciprocal of the ones-column, broadcast multiply

Host-side prep (free; only HW time counts): Q/K transposed to [d, seq] and
packed two-halves-in-128-partitions for full-bandwidth DMA, V packed to the
[128, chunk, 65] ones-augmented layout, output un-permuted from the block
layout the device writes.
"""
import os
import numpy as np
import ml_dtypes

import concourse.bass as bass
import concourse.tile as tile
from concourse import bacc, mybir
from concourse.bass import ds
from concourse.bass_utils import run_bass_kernel_spmd

F32 = mybir.dt.float32
F32R = mybir.dt.float32r
BF16 = mybir.dt.bfloat16

N = 16384
D = 64
NQ = 32          # trunk size
NK = 128         # window size
PAD = (NK - NQ) // 2      # 48 key halo, but the q-col pad is 64 (2 trunks)
C = N // 128     # 128 key chunks
B = C + 1        # 129 query blocks of 128 rows, block b = seq [128b-64, 128b+64)
H_PER_CORE = 2
N_CORES = 8

# qT padded to 16512 cols (64 zero-cols each side); packed halves overlap so
# every chunk's 256-col q-slice stays inside one half.
QT_COLS = 8448   # half A = padded cols [0, 8448), half B = [8064, 16512)
QT_B_OFF = 8064
KT_COLS = 8192   # half A = cols [0, 8192), half B = [8192, 16384)

QK_DTYPE = os.environ.get("QK_DTYPE", "fp16")  # "fp16" | "f32r" | "bf16"

LAST_EXEC_TIME_NS = None
LAST_RESULTS = None


def _build_mask() -> np.ndarray:
    """mask[kk, j] = 1 iff 32g-112 <= kk < 32g+16, g = j//32."""
    kk = np.arange(128)[:, None]
    g = np.arange(256)[None, :] // 32
    valid = (kk >= 32 * g - 112) & (kk < 32 * g + 16)
    return valid.astype(ml_dtypes.bfloat16)


_DT_QK = {"f32r": F32R, "bf16": BF16, "fp16": mybir.dt.float16}
_NP_QK = {"f32r": np.float32, "bf16": ml_dtypes.bfloat16, "fp16": np.float16}


def build_nc():
    dt_qk = _DT_QK[QK_DTYPE]
    nc = bacc.Bacc(None, target_bir_lowering=False)

    qt_ext = nc.declare_dram_parameter("qt", [H_PER_CORE, 128, QT_COLS], dt_qk, isOutput=False)
    kt_ext = nc.declare_dram_parameter("kt", [H_PER_CORE, 128, KT_COLS], dt_qk, isOutput=False)
    v_ext = nc.declare_dram_parameter("v65", [H_PER_CORE, 128, C * 65], BF16, isOutput=False)
    m_ext = nc.declare_dram_parameter("mask", [128, 512], BF16, isOutput=False)
    out_ext = nc.declare_dram_parameter("out", [H_PER_CORE, 128, B * 64], F32, isOutput=True)

    with tile.TileContext(nc) as tc:
        with (
            tc.tile_pool(name="inputs", bufs=2) as inputs,
            tc.tile_pool(name="singles", bufs=1) as singles,
            tc.tile_pool(name="at", bufs=3) as at_pool,
            tc.tile_pool(name="st", bufs=2) as st_pool,
            tc.tile_pool(name="rt", bufs=2) as rt_pool,
            tc.tile_pool(name="ps_s", bufs=3, space="PSUM") as ps_s,
            tc.tile_pool(name="ps_o", bufs=2, space="PSUM") as ps_o,
        ):
            mask_t = singles.tile([128, 512], BF16)
            nc.sync.dma_start(out=mask_t, in_=m_ext[:, :])

            for h in range(H_PER_CORE):
                qt_t = inputs.tile([128, QT_COLS], dt_qk, tag="qt")
                kt_t = inputs.tile([128, KT_COLS], dt_qk, tag="kt")
                v_t = inputs.tile([128, C * 65], BF16, tag="v")
                nc.sync.dma_start(out=qt_t, in_=qt_ext[h])
                nc.sync.dma_start(out=kt_t, in_=kt_ext[h])
                nc.sync.dma_start(out=v_t, in_=v_ext[h])

                po = {}   # batch J -> psum tile [128, 4, 65]
                po_touched = set()  # J values whose tile already has a start=True
                st_t = None
                for u in range(C // 2):
                    # --- QK^T for chunk pair (2u, 2u+1), one psum bank ---
                    s_ps = ps_s.tile([128, 512], F32)
                    for ci in range(2):
                        c = 2 * u + ci
                        if c < 64:
                            lhs = kt_t[0:64, ds(128 * c, 128)]
                            rhs = qt_t[0:64, ds(128 * c, 256)]
                        else:
                            lhs = kt_t[64:128, ds(128 * c - KT_COLS, 128)]
                            rhs = qt_t[64:128, ds(128 * c - QT_B_OFF, 256)]
                        # first matmul's start=True wipes the whole bank, so the
                        # second must ride the cleared has_written bits instead.
                        nc.tensor.matmul(
                            s_ps[:, ds(256 * ci, 256)], lhsT=lhs, rhs=rhs,
                            start=(ci == 0), stop=(ci == 1), skip_group_check=True,
                        )

                    # --- exp + mask, both chunks in one op each ---
                    at_t = at_pool.tile([128, 512], BF16)
                    nc.scalar.activation(out=at_t, in_=s_ps, func=mybir.ActivationFunctionType.Exp)
                    nc.vector.tensor_mul(at_t, at_t, mask_t)

                    for ci in range(2):
                        c = 2 * u + ci
                        ao = 256 * ci
                        # --- AV + rowsum (ones-column in v) ---
                        j_new, j_old = (c + 1) % 4, c % 4
                        J_new, J_old = (c + 1) // 4, c // 4
                        if J_new not in po:
                            po[J_new] = ps_o.tile([128, 4, 65], F32, tag="po", name=f"po_h{h}_{J_new}")
                        if J_old not in po:
                            po[J_old] = ps_o.tile([128, 4, 65], F32, tag="po", name=f"po_h{h}_{J_old}")
                        vslice = v_t[:, ds(65 * c, 65)]
                        # start=True clears the ENTIRE psum bank (has_written
                        # bits), so only the first matmul touching each 4-block
                        # tile may set it; later slots rely on clear has_written
                        # bits to overwrite on first write, accumulate on second.
                        nc.tensor.matmul(
                            po[J_new][:, j_new, :], lhsT=at_t[:, ds(ao + 128, 128)], rhs=vslice,
                            start=(J_new not in po_touched),
                            stop=(c == C - 1), skip_group_check=True,
                        )
                        po_touched.add(J_new)
                        nc.tensor.matmul(
                            po[J_old][:, j_old, :], lhsT=at_t[:, ds(ao, 128)], rhs=vslice,
                            start=(J_old not in po_touched),
                            stop=(j_old == 3), skip_group_check=True,
                        )
                        po_touched.add(J_old)

                        # --- normalize completed batch of 4 blocks ---
                        if c % 4 == 3:
                            J = c // 4            # blocks 4J..4J+3 done
                            if st_t is None:
                                st_t = st_pool.tile([128, 16, 64], F32, tag="st")
                            pj = po.pop(J)
                            rt_t = rt_pool.tile([128, 4], F32)
                            nc.vector.reciprocal(rt_t, pj[:, :, 64:65])
                            rb = bass.AP(
                                tensor=rt_t.tensor, offset=rt_t.offset,
                                ap=[rt_t.ap[0], rt_t.ap[1], [0, 64]],
                            )
                            s4 = J % 4
                            nc.vector.tensor_mul(
                                st_t[:, ds(4 * s4, 4), :], pj[:, :, 0:64], rb,
                            )
                            if J % 4 == 3:
                                G = J // 4        # blocks 16G..16G+15 staged
                                nc.sync.dma_start(
                                    out=out_ext[h][:, ds(1024 * G, 1024)], in_=st_t,
                                )
                                st_t = None

                # --- tail: block 128 (batch 32, slot 0) ---
                pj = po.pop(32)
                rt_t = rt_pool.tile([128, 1], F32)
                nc.vector.reciprocal(rt_t, pj[:, 0, 64:65])
                rb = bass.AP(
                    tensor=rt_t.tensor, offset=rt_t.offset,
                    ap=[rt_t.ap[0], [0, 64]],
                )
                ot = st_pool.tile([128, 64], F32, tag="tail")
                nc.vector.tensor_mul(ot, pj[:, 0, 0:64], rb)
                nc.sync.dma_start(out=out_ext[h][:, ds(64 * 128, 64)], in_=ot)

    nc.finalize()
    return nc


_NC_CACHE = {}


def _get_nc():
    key = QK_DTYPE
    if key not in _NC_CACHE:
        _NC_CACHE[key] = build_nc()
    return _NC_CACHE[key]


def _prep_core(q2: np.ndarray, k2: np.ndarray, v2: np.ndarray, mask: np.ndarray):
    """q2/k2/v2: [2, N, D] f32 for this core's heads -> in_map dict."""
    np_qk = _NP_QK[QK_DTYPE]
    qt = np.zeros((H_PER_CORE, 128, QT_COLS), dtype=np_qk)
    kt = np.empty((H_PER_CORE, 128, KT_COLS), dtype=np_qk)
    v65 = np.empty((H_PER_CORE, 128, C * 65), dtype=ml_dtypes.bfloat16)
    for h in range(H_PER_CORE):
        qT = np.zeros((D, 64 + N + 64), dtype=np.float32)
        qT[:, 64:64 + N] = q2[h].T
        qt[h, 0:64] = qT[:, 0:QT_COLS].astype(np_qk)
        qt[h, 64:128] = qT[:, QT_B_OFF:QT_B_OFF + QT_COLS].astype(np_qk)
        kT = k2[h].T.astype(np_qk)
        kt[h, 0:64] = kT[:, 0:KT_COLS]
        kt[h, 64:128] = kT[:, KT_COLS:N]
        vv = np.ones((128, C, 65), dtype=ml_dtypes.bfloat16)
        vv[:, :, 0:64] = np.transpose(
            v2[h].reshape(C, 128, D), (1, 0, 2)
        ).astype(ml_dtypes.bfloat16)
        v65[h] = vv.reshape(128, C * 65)
    return {"qt": qt, "kt": kt, "v65": v65, "mask": mask}


def kernel(q: np.ndarray, k: np.ndarray, v: np.ndarray) -> np.ndarray:
    global LAST_EXEC_TIME_NS, LAST_RESULTS
    q = np.asarray(q)
    k = np.asarray(k)
    v = np.asarray(v)
    Bq, H = q.shape[0], q.shape[1]
    assert (Bq, H) == (1, 16) and q.shape[2] == N and q.shape[3] == D

    mask = np.tile(_build_mask(), (1, 2))  # [128, 512] for 2-chunk batching
    in_maps = []
    for i in range(N_CORES):
        hs = slice(H_PER_CORE * i, H_PER_CORE * (i + 1))
        in_maps.append(_prep_core(q[0, hs], k[0, hs], v[0, hs], mask))

    nc = _get_nc()
    res = run_bass_kernel_spmd(nc, in_maps, core_ids=list(range(N_CORES)))
    LAST_RESULTS = res
    LAST_EXEC_TIME_NS = res.exec_time_ns

    out = np.empty((1, H, N, D), dtype=np.float32)
    for i in range(N_CORES):
        od = np.asarray(res.results[i]["out"])  # [2, 128, B*64]
        # [2, 128, B, 64] -> [2, B, 128, 64] -> [2, B*128, 64]; rows 64..64+N
        o = od.reshape(H_PER_CORE, 128, B, 64).transpose(0, 2, 1, 3).reshape(
            H_PER_CORE, B * 128, 64
        )
        out[0, H_PER_CORE * i:H_PER_CORE * (i + 1)] = o[:, 64:64 + N, :]
    return out
